# Initial kernel scaffold
#
"""Trainium2 Bass kernel for nn_DistNN_88794153877510 (gnn_message_passing).

Computation (reference):
  atom_1 = relu(atom_feat[:, :128] @ W_ea + b_ea)
  atom_2 = relu(atom_feat[:, 128:] @ W_ea + b_ea)
  x_rdf  = relu6(rdf_feat @ W_er + b_er)
  x_bdf  = relu6(bdf_feat @ W_eb + b_eb)
  h = [ [a1,a2,x_rdf] @ W_fr + b_fr | [a1,a2,x_bdf] @ W_fb + b_fb ]   # [E,256]
  h = relu(batchnorm(h))           (training stats over all E rows)
  pooled = segment_mean(h, graph_idx, G)                              # [G,256]
  z = relu([pooled, ref_feat] @ W1 + b1); gap = relu(z @ W2 + b2)     # [G,1]

Distribution: shard whole graphs across the 8 cores (128 graphs/core; rows of
core k = rows with graph_idx in [128k, 128k+128)), pad every core to a common
row count R. BN statistics are computed per-core and AllReduced (with an exact
host-side correction for the zero-input pad rows); per-graph segment sums are
computed locally via one-hot matmuls (sorted graph_idx means graphs never
cross cores), pooled means are AllGathered and the tiny final MLP is computed
redundantly on every core.

Two passes over the edge rows with a bf16 spill of pre-BN h to HBM between
them (BN needs global stats before the nonlinear relu -> segment sum).
All big matmuls run in bf16 with f32 PSUM accumulation.
"""

import numpy as np
import ml_dtypes

import concourse.bass as bass
import concourse.mybir as mybir
import concourse.tile as tile
from concourse import bacc
from concourse.bass_utils import run_bass_kernel_spmd

BF16 = ml_dtypes.bfloat16
F32 = mybir.dt.float32
BF = mybir.dt.bfloat16

N_CORES = 8
N_AF = 128
EPS = 1e-5

# ---------------------------------------------------------------------------
# host-side preprocessing
# ---------------------------------------------------------------------------


def _to_bf(x):
    return np.asarray(x, dtype=np.float32).astype(BF16)


def _host_prep(inputs):
    atom = np.asarray(inputs["atom_feat"], dtype=np.float32)
    rdf = np.asarray(inputs["rdf_feat"], dtype=np.float32)
    bdf = np.asarray(inputs["bdf_feat"], dtype=np.float32)
    gidx = np.asarray(inputs["graph_idx"]).astype(np.int64)
    ref = np.asarray(inputs["ref_feat"], dtype=np.float32)
    E = atom.shape[0]
    G = ref.shape[0]
    GPC = G // N_CORES

    bounds = np.searchsorted(gidx, np.arange(0, G + 1, GPC), side="left")
    rows_per_core = bounds[1:] - bounds[:-1]
    R = int(max(1024, -(-int(rows_per_core.max()) // 1024) * 1024))
    NSB = R // 1024
    T_pad = N_CORES * R - E

    cnt = np.bincount(gidx, minlength=G).astype(np.float32)
    recip = (1.0 / np.maximum(cnt, 1.0)).astype(np.float32)

    # weights (shared across cores)
    W_ea = np.asarray(inputs["W_ea"], np.float32)
    W_er = np.asarray(inputs["W_er"], np.float32)
    W_eb = np.asarray(inputs["W_eb"], np.float32)
    W_fr = np.asarray(inputs["W_fr"], np.float32)
    W_fb = np.asarray(inputs["W_fb"], np.float32)
    b_ea = np.asarray(inputs["b_ea"], np.float32)
    b_er = np.asarray(inputs["b_er"], np.float32)
    b_eb = np.asarray(inputs["b_eb"], np.float32)
    b_fr = np.asarray(inputs["b_fr"], np.float32)
    b_fb = np.asarray(inputs["b_fb"], np.float32)
    gam = np.asarray(inputs["bn_gamma"], np.float32)
    bet = np.asarray(inputs["bn_beta"], np.float32)
    W1 = np.asarray(inputs["W1"], np.float32)
    b1 = np.asarray(inputs["b1"], np.float32)
    W2 = np.asarray(inputs["W2"], np.float32)
    b2 = np.asarray(inputs["b2"], np.float32)

    W_ea_b = _to_bf(W_ea)
    W_er_b = _to_bf(W_er)
    W_eb_b = _to_bf(W_eb)
    W_fr_b = _to_bf(W_fr)
    W_fb_b = _to_bf(W_fb)

    # h value of an all-zero (pad) row, replicating device bf16 arithmetic:
    # a1 = bf16(relu(b_ea)); xr = bf16(min(b_er,6)) then max(.,0); second layer
    # in f32 from bf16 operands.
    a1p = np.maximum(b_ea, 0.0).astype(BF16).astype(np.float32)
    xrp = np.maximum(np.minimum(b_er, 6.0).astype(BF16).astype(np.float32), 0.0)
    xbp = np.maximum(np.minimum(b_eb, 6.0).astype(BF16).astype(np.float32), 0.0)
    h1p = np.concatenate([a1p, a1p, xrp]).astype(BF16).astype(np.float32)
    h2p = np.concatenate([a1p, a1p, xbp]).astype(BF16).astype(np.float32)
    hpad = np.concatenate(
        [h1p @ W_fr_b.astype(np.float32) + b_fr, h2p @ W_fb_b.astype(np.float32) + b_fb]
    ).astype(np.float32)  # [256]

    def fm2(v):  # [256] -> [128, 2] feature-major halves
        return np.ascontiguousarray(v.reshape(2, 128).T).astype(np.float32)

    shared = {
        "w_ea": W_ea_b,
        "w_er": W_er_b,
        "w_eb": W_eb_b,
        # [128, 3, 128]: chunk j of the 384-dim contraction as lhsT
        "w_fr": np.ascontiguousarray(W_fr_b.reshape(3, 128, 128).transpose(1, 0, 2)),
        "w_fb": np.ascontiguousarray(W_fb_b.reshape(3, 128, 128).transpose(1, 0, 2)),
        "b_ea_c": b_ea.reshape(128, 1),
        "b_er_c": b_er.reshape(128, 1),
        "b_eb_c": b_eb.reshape(128, 1),
        "b_fr_c": b_fr.reshape(128, 1),
        "b_fb_c": b_fb.reshape(128, 1),
        "gam_fm": fm2(gam),
        "bet_fm": fm2(bet),
        "hpad_fm": fm2(hpad),
        "hpad2_fm": fm2(hpad * hpad),
        "iota_t": np.broadcast_to(
            np.arange(128, dtype=np.float32), (128, 128)
        ).copy(),
        "ones_r": np.ones((1, 128), dtype=BF16),
        # W1: [257, 16] -> chunks [128, 2, 16] + ref row [1, 16]
        "w1c": np.ascontiguousarray(
            _to_bf(W1[:256]).reshape(2, 128, 16).transpose(1, 0, 2)
        ),
        "w1r": _to_bf(W1[256:257]),
        "b1_c": b1.reshape(16, 1),
        "w2": _to_bf(W2),
        "b2_c": b2.reshape(1, 1),
        "refT": _to_bf(ref.reshape(1, G)),
    }

    def shard(x_full, width, core):
        lo, hi = bounds[core], bounds[core + 1]
        out = np.zeros((R, width), dtype=np.float32)
        out[: hi - lo] = x_full[lo:hi]
        # [R, w] -> [NSB, 128, 8, w]
        return np.ascontiguousarray(
            out.reshape(NSB, 8, 128, width).transpose(0, 2, 1, 3)
        )

    in_maps = []
    for c in range(N_CORES):
        lo, hi = bounds[c], bounds[c + 1]
        idxl = np.full((R,), -1.0, dtype=np.float32)
        idxl[: hi - lo] = (gidx[lo:hi] - c * GPC).astype(np.float32)
        m = dict(shared)
        m["atom_r"] = shard(atom, 2 * N_AF, c)
        m["rdf_r"] = shard(rdf, 128, c)
        m["bdf_r"] = shard(bdf, 128, c)
        m["idx_r"] = np.ascontiguousarray(
            idxl.reshape(NSB, 8, 128).transpose(0, 2, 1)
        )
        m["recip_c"] = recip[c * GPC : (c + 1) * GPC].reshape(GPC, 1)
        in_maps.append(m)

    params = dict(E=E, G=G, GPC=GPC, R=R, NSB=NSB, T_pad=T_pad)
    return params, in_maps


# ---------------------------------------------------------------------------
# device program
# ---------------------------------------------------------------------------


def _build(nc, p):
    E, G, GPC, R, NSB = p["E"], p["G"], p["GPC"], p["R"], p["NSB"]
    T_pad = p["T_pad"]
    add = mybir.AluOpType.add
    sub = mybir.AluOpType.subtract
    mult = mybir.AluOpType.mult
    mn = mybir.AluOpType.min
    mx = mybir.AluOpType.max
    iseq = mybir.AluOpType.is_equal
    Relu = mybir.ActivationFunctionType.Relu
    Ident = mybir.ActivationFunctionType.Identity
    Square = mybir.ActivationFunctionType.Square
    Sqrt = mybir.ActivationFunctionType.Sqrt
    AX = mybir.AxisListType.X

    # ---- I/O -------------------------------------------------------------
    atom_d = nc.dram_tensor("atom_r", [NSB, 128, 8, 256], F32, kind="ExternalInput")
    rdf_d = nc.dram_tensor("rdf_r", [NSB, 128, 8, 128], F32, kind="ExternalInput")
    bdf_d = nc.dram_tensor("bdf_r", [NSB, 128, 8, 128], F32, kind="ExternalInput")
    idx_d = nc.dram_tensor("idx_r", [NSB, 128, 8], F32, kind="ExternalInput")
    recip_d = nc.dram_tensor("recip_c", [GPC, 1], F32, kind="ExternalInput")
    wea_d = nc.dram_tensor("w_ea", [128, 128], BF, kind="ExternalInput")
    wer_d = nc.dram_tensor("w_er", [128, 128], BF, kind="ExternalInput")
    web_d = nc.dram_tensor("w_eb", [128, 128], BF, kind="ExternalInput")
    wfr_d = nc.dram_tensor("w_fr", [128, 3, 128], BF, kind="ExternalInput")
    wfb_d = nc.dram_tensor("w_fb", [128, 3, 128], BF, kind="ExternalInput")
    bea_d = nc.dram_tensor("b_ea_c", [128, 1], F32, kind="ExternalInput")
    ber_d = nc.dram_tensor("b_er_c", [128, 1], F32, kind="ExternalInput")
    beb_d = nc.dram_tensor("b_eb_c", [128, 1], F32, kind="ExternalInput")
    bfr_d = nc.dram_tensor("b_fr_c", [128, 1], F32, kind="ExternalInput")
    bfb_d = nc.dram_tensor("b_fb_c", [128, 1], F32, kind="ExternalInput")
    gam_d = nc.dram_tensor("gam_fm", [128, 2], F32, kind="ExternalInput")
    bet_d = nc.dram_tensor("bet_fm", [128, 2], F32, kind="ExternalInput")
    hpad_d = nc.dram_tensor("hpad_fm", [128, 2], F32, kind="ExternalInput")
    hpad2_d = nc.dram_tensor("hpad2_fm", [128, 2], F32, kind="ExternalInput")
    iota_d = nc.dram_tensor("iota_t", [128, 128], F32, kind="ExternalInput")
    ones_d = nc.dram_tensor("ones_r", [1, 128], BF, kind="ExternalInput")
    w1c_d = nc.dram_tensor("w1c", [128, 2, 16], BF, kind="ExternalInput")
    w1r_d = nc.dram_tensor("w1r", [1, 16], BF, kind="ExternalInput")
    b1_d = nc.dram_tensor("b1_c", [16, 1], F32, kind="ExternalInput")
    w2_d = nc.dram_tensor("w2", [16, 1], BF, kind="ExternalInput")
    b2_d = nc.dram_tensor("b2_c", [1, 1], F32, kind="ExternalInput")
    refT_d = nc.dram_tensor("refT", [1, G], BF, kind="ExternalInput")
    out_d = nc.dram_tensor("gap_t", [1, G], F32, kind="ExternalOutput")

    with tile.TileContext(nc) as tc:
        import contextlib

        with contextlib.ExitStack() as S:
            consts = S.enter_context(tc.tile_pool(name="consts", bufs=1))
            statsp = S.enter_context(tc.tile_pool(name="stats", bufs=1))
            dram = S.enter_context(tc.tile_pool(name="dram", bufs=1, space="DRAM"))

            # persistent DRAM scratch
            hb0 = dram.tile([128, R], BF, tag="hb0")
            hb1 = dram.tile([128, R], BF, tag="hb1")
            st_in = dram.tile([128, 4], F32, tag="st_in")
            st_out = dram.tile([128, 4], F32, tag="st_out")
            scl_d = dram.tile([256], F32, tag="scl_d")
            bia_d = dram.tile([256], F32, tag="bia_d")
            ag_in = dram.tile([GPC, 256], BF, tag="ag_in")
            ag_out = dram.tile([G, 256], BF, tag="ag_out")

            # constants in SBUF
            def cload(dt_, handle, shape, name):
                t = consts.tile(shape, dt_, tag=name)
                nc.sync.dma_start(t[:], handle[:])
                return t

            wea = cload(BF, wea_d, [128, 128], "wea")
            wer = cload(BF, wer_d, [128, 128], "wer")
            web = cload(BF, web_d, [128, 128], "web")
            wfr = cload(BF, wfr_d, [128, 3, 128], "wfr")
            wfb = cload(BF, wfb_d, [128, 3, 128], "wfb")
            bea = cload(F32, bea_d, [128, 1], "bea")
            ber = cload(F32, ber_d, [128, 1], "ber")
            beb = cload(F32, beb_d, [128, 1], "beb")
            bfr = cload(F32, bfr_d, [128, 1], "bfr")
            bfb = cload(F32, bfb_d, [128, 1], "bfb")
            gam = cload(F32, gam_d, [128, 2], "gam")
            bet = cload(F32, bet_d, [128, 2], "bet")
            hpad = cload(F32, hpad_d, [128, 2], "hpad")
            hpad2 = cload(F32, hpad2_d, [128, 2], "hpad2")
            iota = cload(F32, iota_d, [128, 128], "iota")
            ones_r = cload(BF, ones_d, [1, 128], "ones")
            w1c = cload(BF, w1c_d, [128, 2, 16], "w1c")
            w1r = cload(BF, w1r_d, [1, 16], "w1r")
            b1c = cload(F32, b1_d, [16, 1], "b1c")
            w2 = cload(BF, w2_d, [16, 1], "w2")
            b2c = cload(F32, b2_d, [1, 1], "b2c")
            refT = cload(BF, refT_d, [1, G], "refT")
            recip = cload(F32, recip_d, [GPC, 1], "recip")

            # BN affine broadcast tiles (filled after the stats AllReduce)
            scaleB = consts.tile([128, 256], BF, tag="scaleB")
            biasB = consts.tile([128, 256], BF, tag="biasB")
            out_sb = consts.tile([1, G], F32, tag="out_sb")

            # per-block stat partials
            ssum0 = statsp.tile([128, 2 * NSB], F32, tag="ssum0")
            ssum1 = statsp.tile([128, 2 * NSB], F32, tag="ssum1")
            ssq0 = statsp.tile([128, 2 * NSB], F32, tag="ssq0")
            ssq1 = statsp.tile([128, 2 * NSB], F32, tag="ssq1")

            # ================= PASS 1 =================
            with contextlib.ExitStack() as S1:
                pAf = S1.enter_context(tc.tile_pool(name="p1_af", bufs=2))
                pRf = S1.enter_context(tc.tile_pool(name="p1_rf", bufs=2))
                pAb = S1.enter_context(tc.tile_pool(name="p1_ab", bufs=2))
                pT = S1.enter_context(tc.tile_pool(name="p1_t", bufs=2))
                pE = S1.enter_context(tc.tile_pool(name="p1_e", bufs=2))
                pS = S1.enter_context(tc.tile_pool(name="p1_s", bufs=3))
                ps1 = S1.enter_context(
                    tc.tile_pool(name="ps_l1", bufs=4, space="PSUM")
                )
                ps2 = S1.enter_context(
                    tc.tile_pool(name="ps_l2", bufs=3, space="PSUM")
                )

                for sb in range(NSB):
                    af = pAf.tile([128, 8, 256], F32, tag="af")
                    rf = pRf.tile([128, 8, 128], F32, tag="rf")
                    bf = pRf.tile([128, 8, 128], F32, tag="bf")
                    nc.sync.dma_start(af[:], atom_d[sb])
                    nc.sync.dma_start(rf[:], rdf_d[sb])
                    nc.sync.dma_start(bf[:], bdf_d[sb])

                    ab = pAb.tile([128, 8, 256], BF, tag="ab")
                    rb = pAb.tile([128, 8, 128], BF, tag="rb")
                    bb = pAb.tile([128, 8, 128], BF, tag="bb")
                    nc.gpsimd.tensor_copy(ab[:], af[:])
                    nc.gpsimd.tensor_copy(rb[:], rf[:])
                    nc.gpsimd.tensor_copy(bb[:], bf[:])

                    # transpose to feature-major [feat, rows]
                    aT = pT.tile([128, 2, 8, 128], BF, tag="aT")
                    rT = pT.tile([128, 8, 128], BF, tag="rT")
                    bT = pT.tile([128, 8, 128], BF, tag="bT")
                    for g in range(8):
                        for h in range(2):
                            nc.sync.dma_start(
                                aT[:, h, g, :],
                                ab[:, g, h * 128 : (h + 1) * 128],
                                transpose=True,
                            )
                        nc.sync.dma_start(rT[:, g, :], rb[:, g, :], transpose=True)
                        nc.sync.dma_start(bT[:, g, :], bb[:, g, :], transpose=True)

                    for blk in range(2):
                        gs = slice(blk * 4, blk * 4 + 4)
                        pa1 = ps1.tile([128, 512], F32, tag="l1")
                        pa2 = ps1.tile([128, 512], F32, tag="l1")
                        pr = ps1.tile([128, 512], F32, tag="l1")
                        pb = ps1.tile([128, 512], F32, tag="l1")
                        nc.tensor.matmul(pa1[:], wea[:], aT[:, 0, gs, :])
                        nc.tensor.matmul(pa2[:], wea[:], aT[:, 1, gs, :])
                        nc.tensor.matmul(pr[:], wer[:], rT[:, gs, :])
                        nc.tensor.matmul(pb[:], web[:], bT[:, gs, :])

                        a1 = pE.tile([128, 512], BF, tag="a1")
                        a2 = pE.tile([128, 512], BF, tag="a2")
                        xr = pE.tile([128, 512], BF, tag="xr")
                        xb = pE.tile([128, 512], BF, tag="xb")
                        t6r = pE.tile([128, 512], BF, tag="t6r")
                        t6b = pE.tile([128, 512], BF, tag="t6b")
                        nc.scalar.activation(a1[:], pa1[:], Relu, bias=bea[:])
                        nc.scalar.activation(a2[:], pa2[:], Relu, bias=bea[:])
                        nc.vector.tensor_scalar(t6r[:], pr[:], ber[:], 6.0, add, mn)
                        nc.vector.tensor_scalar(xr[:], t6r[:], 0.0, None, mx)
                        nc.vector.tensor_scalar(t6b[:], pb[:], beb[:], 6.0, add, mn)
                        nc.vector.tensor_scalar(xb[:], t6b[:], 0.0, None, mx)

                        pfr = ps2.tile([128, 512], F32, tag="l2")
                        pfb = ps2.tile([128, 512], F32, tag="l2")
                        for j, src in enumerate([a1, a2, xr]):
                            nc.tensor.matmul(
                                pfr[:], wfr[:, j, :], src[:],
                                start=(j == 0), stop=(j == 2),
                            )
                        for j, src in enumerate([a1, a2, xb]):
                            nc.tensor.matmul(
                                pfb[:], wfb[:, j, :], src[:],
                                start=(j == 0), stop=(j == 2),
                            )

                        col = 2 * sb + blk
                        sp0 = pS.tile([128, 512], BF, tag="sp0")
                        sp1 = pS.tile([128, 512], BF, tag="sp1")
                        sq0 = pS.tile([128, 512], BF, tag="sq0")
                        sq1 = pS.tile([128, 512], BF, tag="sq1")
                        nc.scalar.activation(
                            sp0[:], pfr[:], Ident, bias=bfr[:],
                            accum_out=ssum0[:, col : col + 1],
                        )
                        nc.scalar.activation(
                            sq0[:], pfr[:], Square, bias=bfr[:],
                            accum_out=ssq0[:, col : col + 1],
                        )
                        nc.scalar.activation(
                            sp1[:], pfb[:], Ident, bias=bfb[:],
                            accum_out=ssum1[:, col : col + 1],
                        )
                        nc.scalar.activation(
                            sq1[:], pfb[:], Square, bias=bfb[:],
                            accum_out=ssq1[:, col : col + 1],
                        )
                        off = sb * 1024 + blk * 512
                        nc.sync.dma_start(hb0[:, off : off + 512], sp0[:])
                        nc.sync.dma_start(hb1[:, off : off + 512], sp1[:])

            # ================= STATS + ALLREDUCE =================
            with contextlib.ExitStack() as SS:
                sp = SS.enter_context(tc.tile_pool(name="stats2", bufs=1))
                psb = SS.enter_context(
                    tc.tile_pool(name="ps_bc", bufs=1, space="PSUM")
                )
                stat = sp.tile([128, 4], F32, tag="stat")
                nc.vector.tensor_reduce(stat[:, 0:1], ssum0[:], AX, add)
                nc.vector.tensor_reduce(stat[:, 1:2], ssum1[:], AX, add)
                nc.vector.tensor_reduce(stat[:, 2:3], ssq0[:], AX, add)
                nc.vector.tensor_reduce(stat[:, 3:4], ssq1[:], AX, add)
                nc.sync.dma_start(st_in[:], stat[:])
                nc.gpsimd.collective_compute(
                    "AllReduce",
                    add,
                    replica_groups=[list(range(N_CORES))],
                    ins=[st_in.opt()],
                    outs=[st_out.opt()],
                )
                statg = sp.tile([128, 4], F32, tag="statg")
                nc.sync.dma_start(statg[:], st_out[:])

                # mu = S/E - hpad*(T_pad/E); msq = S2/E - hpad2*(T_pad/E)
                t1 = sp.tile([128, 2], F32, tag="t1")
                t2 = sp.tile([128, 2], F32, tag="t2")
                muv = sp.tile([128, 2], F32, tag="muv")
                msq = sp.tile([128, 2], F32, tag="msq")
                var = sp.tile([128, 2], F32, tag="var")
                sd = sp.tile([128, 2], F32, tag="sd")
                rstd = sp.tile([128, 2], F32, tag="rstd")
                scl = sp.tile([128, 2], F32, tag="scl")
                bia = sp.tile([128, 2], F32, tag="bia")
                nc.vector.tensor_scalar(t1[:], statg[:, 0:2], 1.0 / E, None, mult)
                nc.vector.tensor_scalar(t2[:], hpad[:], T_pad / E, None, mult)
                nc.vector.tensor_tensor(muv[:], t1[:], t2[:], sub)
                nc.vector.tensor_scalar(t1[:], statg[:, 2:4], 1.0 / E, None, mult)
                nc.vector.tensor_scalar(t2[:], hpad2[:], T_pad / E, None, mult)
                nc.vector.tensor_tensor(msq[:], t1[:], t2[:], sub)
                nc.vector.tensor_tensor(t1[:], muv[:], muv[:], mult)
                nc.vector.tensor_tensor(var[:], msq[:], t1[:], sub)
                nc.scalar.activation(sd[:], var[:], Sqrt, bias=EPS)
                nc.vector.reciprocal(rstd[:], sd[:])
                nc.vector.tensor_tensor(scl[:], gam[:], rstd[:], mult)
                nc.vector.tensor_tensor(t2[:], muv[:], scl[:], mult)
                nc.vector.tensor_tensor(bia[:], bet[:], t2[:], sub)

                # feature-major [128,2] -> DRAM [256] -> row [1,256] -> bcast
                nc.sync.dma_start(scl_d.opt().rearrange("(j p) -> p j", p=128), scl[:])
                nc.sync.dma_start(bia_d.opt().rearrange("(j p) -> p j", p=128), bia[:])
                srow_f = sp.tile([1, 256], F32, tag="srow_f")
                brow_f = sp.tile([1, 256], F32, tag="brow_f")
                nc.sync.dma_start(srow_f[:], scl_d.opt().rearrange("(j f) -> j f", j=1))
                nc.sync.dma_start(brow_f[:], bia_d.opt().rearrange("(j f) -> j f", j=1))
                srow = sp.tile([1, 256], BF, tag="srow")
                brow = sp.tile([1, 256], BF, tag="brow")
                nc.vector.tensor_copy(srow[:], srow_f[:])
                nc.vector.tensor_copy(brow[:], brow_f[:])
                pbc = psb.tile([128, 256], F32, tag="bc")
                nc.tensor.matmul(pbc[:], ones_r[:], srow[:])
                nc.scalar.activation(scaleB[:], pbc[:], Ident)
                pbc2 = psb.tile([128, 256], F32, tag="bc")
                nc.tensor.matmul(pbc2[:], ones_r[:], brow[:])
                nc.scalar.activation(biasB[:], pbc2[:], Ident)

            # ================= PASS 2 =================
            with contextlib.ExitStack() as S2:
                p2h = S2.enter_context(tc.tile_pool(name="p2_h", bufs=4))
                p2b = S2.enter_context(tc.tile_pool(name="p2_b", bufs=4))
                p2i = S2.enter_context(tc.tile_pool(name="p2_i", bufs=2))
                psg = S2.enter_context(
                    tc.tile_pool(name="ps_seg", bufs=1, space="PSUM")
                )
                seg = psg.tile([GPC, 256], F32, tag="seg")
                n_groups = NSB * 8
                gi = 0
                for sb in range(NSB):
                    idxt = p2i.tile([128, 8], F32, tag="idx")
                    nc.sync.dma_start(idxt[:], idx_d[sb])
                    for g in range(8):
                        off = sb * 1024 + g * 128
                        hrm = p2h.tile([128, 256], BF, tag="hrm")
                        nc.sync.dma_start(
                            hrm[:, 0:128], hb0[:, off : off + 128], transpose=True
                        )
                        nc.sync.dma_start(
                            hrm[:, 128:256], hb1[:, off : off + 128], transpose=True
                        )
                        tt = p2b.tile([128, 256], BF, tag="tt")
                        uu = p2b.tile([128, 256], BF, tag="uu")
                        hn = p2b.tile([128, 256], BF, tag="hn")
                        oh = p2b.tile([128, GPC], BF, tag="oh")
                        nc.vector.tensor_tensor(tt[:], hrm[:], scaleB[:], mult)
                        nc.vector.tensor_tensor(uu[:], tt[:], biasB[:], add)
                        nc.gpsimd.tensor_scalar(hn[:], uu[:], 0.0, None, mx)
                        nc.gpsimd.tensor_scalar(
                            oh[:], iota[:], idxt[:, g : g + 1], None, iseq
                        )
                        nc.tensor.matmul(
                            seg[:], oh[:], hn[:],
                            start=(gi == 0), stop=(gi == n_groups - 1),
                        )
                        gi += 1

                # pooled means + AllGather
                pooled = p2h.tile([GPC, 256], BF, tag="pooled")
                nc.vector.tensor_scalar(pooled[:], seg[:], recip[:], None, mult)
                nc.sync.dma_start(ag_in[:], pooled[:])
                nc.gpsimd.collective_compute(
                    "AllGather",
                    mybir.AluOpType.bypass,
                    replica_groups=[list(range(N_CORES))],
                    ins=[ag_in.opt()],
                    outs=[ag_out.opt()],
                )

            # ================= FINAL MLP (replicated) =================
            with contextlib.ExitStack() as SF:
                fp = SF.enter_context(tc.tile_pool(name="fin", bufs=1))
                psf = SF.enter_context(
                    tc.tile_pool(name="ps_fin", bufs=2, space="PSUM")
                )
                nblk = G // 128
                pT0 = fp.tile([128, nblk, 128], BF, tag="pT0")
                pT1 = fp.tile([128, nblk, 128], BF, tag="pT1")
                for k in range(nblk):
                    nc.sync.dma_start(
                        pT0[:, k, :],
                        ag_out[k * 128 : (k + 1) * 128, 0:128],
                        transpose=True,
                    )
                    nc.sync.dma_start(
                        pT1[:, k, :],
                        ag_out[k * 128 : (k + 1) * 128, 128:256],
                        transpose=True,
                    )
                zt = fp.tile([16, G], BF, tag="zt")
                nh = G // 512
                for hh in range(nh):
                    zp = psf.tile([16, 512], F32, tag="zp")
                    gsl = slice(hh * 4, hh * 4 + 4)
                    nc.tensor.matmul(
                        zp[:], w1c[:, 0, :], pT0[:, gsl, :], start=True, stop=False
                    )
                    nc.tensor.matmul(
                        zp[:], w1c[:, 1, :], pT1[:, gsl, :], start=False, stop=False
                    )
                    nc.tensor.matmul(
                        zp[:], w1r[:], refT[:, hh * 512 : (hh + 1) * 512],
                        start=False, stop=True,
                    )
                    nc.scalar.activation(
                        zt[:, hh * 512 : (hh + 1) * 512], zp[:], Relu, bias=b1c[:]
                    )
                for hh in range(nh):
                    gp = psf.tile([1, 512], F32, tag="gp")
                    nc.tensor.matmul(gp[:], w2[:], zt[:, hh * 512 : (hh + 1) * 512])
                    nc.scalar.activation(
                        out_sb[:, hh * 512 : (hh + 1) * 512], gp[:], Relu, bias=b2c[:]
                    )
                nc.sync.dma_start(out_d[:], out_sb[:])

    return nc


# ---------------------------------------------------------------------------
# entry point
# ---------------------------------------------------------------------------

_cache = {}


def _get_program(params_key, params):
    if params_key not in _cache:
        nc = bacc.Bacc(
            "TRN2", target_bir_lowering=False, debug=False, num_devices=N_CORES
        )
        _build(nc, params)
        nc.compile()
        _cache[params_key] = nc
    return _cache[params_key]


def kernel(**inputs) -> np.ndarray:
    params, in_maps = _host_prep(inputs)
    nc = _get_program((params["R"], params["G"], params["E"]), params)
    res = run_bass_kernel_spmd(nc, in_maps, core_ids=list(range(N_CORES)))
    gap = np.asarray(res.results[0]["gap_t"], dtype=np.float32).reshape(-1, 1)
    return gap


# revision 6
# speedup vs baseline: 1.3671x; 1.3671x over previous
"""Trainium2 Bass kernel for nn_DistNN_88794153877510 (gnn_message_passing).

Computation (reference):
  atom_1 = relu(atom_feat[:, :128] @ W_ea + b_ea)
  atom_2 = relu(atom_feat[:, 128:] @ W_ea + b_ea)
  x_rdf  = relu6(rdf_feat @ W_er + b_er)
  x_bdf  = relu6(bdf_feat @ W_eb + b_eb)
  h = [ [a1,a2,x_rdf] @ W_fr + b_fr | [a1,a2,x_bdf] @ W_fb + b_fb ]   # [E,256]
  h = relu(batchnorm(h))           (training stats over all E rows)
  pooled = segment_mean(h, graph_idx, G)                              # [G,256]
  z = relu([pooled, ref_feat] @ W1 + b1); gap = relu(z @ W2 + b2)     # [G,1]

Distribution: shard whole graphs across the 8 cores (128 graphs/core; rows of
core k = rows with graph_idx in [128k, 128k+128)), pad every core to a common
row count R. BN statistics are computed per-core and AllReduced (with an exact
host-side correction for the zero-input pad rows); per-graph segment sums are
computed locally via one-hot matmuls (sorted graph_idx means graphs never
cross cores), pooled means are AllGathered and the tiny final MLP is computed
redundantly on every core.

Two passes over the edge rows with a bf16 spill of pre-BN h to HBM between
them (BN needs global stats before the nonlinear relu -> segment sum).
All big matmuls run in bf16 with f32 PSUM accumulation.
"""

import numpy as np
import ml_dtypes

import concourse.bass as bass
import concourse.mybir as mybir
import concourse.tile as tile
from concourse import bacc
from concourse.bass_utils import run_bass_kernel_spmd

BF16 = ml_dtypes.bfloat16
F32 = mybir.dt.float32
BF = mybir.dt.bfloat16

N_CORES = 8
N_AF = 128
EPS = 1e-5

# ---------------------------------------------------------------------------
# host-side preprocessing
# ---------------------------------------------------------------------------


def _to_bf(x):
    return np.asarray(x, dtype=np.float32).astype(BF16)


def _host_prep(inputs):
    atom = np.asarray(inputs["atom_feat"], dtype=np.float32)
    rdf = np.asarray(inputs["rdf_feat"], dtype=np.float32)
    bdf = np.asarray(inputs["bdf_feat"], dtype=np.float32)
    gidx = np.asarray(inputs["graph_idx"]).astype(np.int64)
    ref = np.asarray(inputs["ref_feat"], dtype=np.float32)
    E = atom.shape[0]
    G = ref.shape[0]
    GPC = G // N_CORES

    bounds = np.searchsorted(gidx, np.arange(0, G + 1, GPC), side="left")
    rows_per_core = bounds[1:] - bounds[:-1]
    R = int(max(1024, -(-int(rows_per_core.max()) // 1024) * 1024))
    NSB = R // 1024
    T_pad = N_CORES * R - E

    cnt = np.bincount(gidx, minlength=G).astype(np.float32)
    recip = (1.0 / np.maximum(cnt, 1.0)).astype(np.float32)

    # weights (shared across cores)
    W_ea = np.asarray(inputs["W_ea"], np.float32)
    W_er = np.asarray(inputs["W_er"], np.float32)
    W_eb = np.asarray(inputs["W_eb"], np.float32)
    W_fr = np.asarray(inputs["W_fr"], np.float32)
    W_fb = np.asarray(inputs["W_fb"], np.float32)
    b_ea = np.asarray(inputs["b_ea"], np.float32)
    b_er = np.asarray(inputs["b_er"], np.float32)
    b_eb = np.asarray(inputs["b_eb"], np.float32)
    b_fr = np.asarray(inputs["b_fr"], np.float32)
    b_fb = np.asarray(inputs["b_fb"], np.float32)
    gam = np.asarray(inputs["bn_gamma"], np.float32)
    bet = np.asarray(inputs["bn_beta"], np.float32)
    W1 = np.asarray(inputs["W1"], np.float32)
    b1 = np.asarray(inputs["b1"], np.float32)
    W2 = np.asarray(inputs["W2"], np.float32)
    b2 = np.asarray(inputs["b2"], np.float32)

    W_ea_b = _to_bf(W_ea)
    W_er_b = _to_bf(W_er)
    W_eb_b = _to_bf(W_eb)
    W_fr_b = _to_bf(W_fr)
    W_fb_b = _to_bf(W_fb)

    # h value of an all-zero (pad) row, replicating device bf16 arithmetic:
    # a1 = bf16(relu(b_ea)); xr = bf16(min(b_er,6)) then max(.,0); second layer
    # in f32 from bf16 operands.
    a1p = np.maximum(b_ea, 0.0).astype(BF16).astype(np.float32)
    xrp = np.maximum(np.minimum(b_er, 6.0).astype(BF16).astype(np.float32), 0.0)
    xbp = np.maximum(np.minimum(b_eb, 6.0).astype(BF16).astype(np.float32), 0.0)
    h1p = np.concatenate([a1p, a1p, xrp]).astype(BF16).astype(np.float32)
    h2p = np.concatenate([a1p, a1p, xbp]).astype(BF16).astype(np.float32)
    hpad = np.concatenate(
        [h1p @ W_fr_b.astype(np.float32) + b_fr, h2p @ W_fb_b.astype(np.float32) + b_fb]
    ).astype(np.float32)  # [256]

    def fm2(v):  # [256] -> [128, 2] feature-major halves
        return np.ascontiguousarray(v.reshape(2, 128).T).astype(np.float32)

    shared = {
        "w_ea": W_ea_b,
        "w_er": W_er_b,
        "w_eb": W_eb_b,
        # [128, 3, 128]: chunk j of the 384-dim contraction as lhsT
        "w_fr": np.ascontiguousarray(W_fr_b.reshape(3, 128, 128).transpose(1, 0, 2)),
        "w_fb": np.ascontiguousarray(W_fb_b.reshape(3, 128, 128).transpose(1, 0, 2)),
        "b_ea_c": b_ea.reshape(128, 1),
        "b_er_c": b_er.reshape(128, 1),
        "b_eb_c": b_eb.reshape(128, 1),
        "b_fr_c": b_fr.reshape(128, 1),
        "b_fb_c": b_fb.reshape(128, 1),
        "gam_fm": fm2(gam),
        "bet_fm": fm2(bet),
        "hpad_fm": fm2(hpad),
        "hpad2_fm": fm2(hpad * hpad),
        "iota_t": np.broadcast_to(
            np.arange(128, dtype=np.float32), (128, 128)
        ).copy(),
        "ones_r": np.ones((1, 128), dtype=BF16),
        # W1: [257, 16] -> chunks [128, 2, 16] + ref row [1, 16]
        "w1c": np.ascontiguousarray(
            _to_bf(W1[:256]).reshape(2, 128, 16).transpose(1, 0, 2)
        ),
        "w1r": _to_bf(W1[256:257]),
        "b1_c": b1.reshape(16, 1),
        "w2": _to_bf(W2),
        "b2_c": b2.reshape(1, 1),
        "refT": _to_bf(ref.reshape(1, G)),
    }

    def shard(x_full, width, core):
        lo, hi = bounds[core], bounds[core + 1]
        out = np.zeros((R, width), dtype=np.float32)
        out[: hi - lo] = x_full[lo:hi]
        # [R, w] -> [NSB, 128, 8, w]
        return np.ascontiguousarray(
            out.reshape(NSB, 8, 128, width).transpose(0, 2, 1, 3)
        )

    in_maps = []
    for c in range(N_CORES):
        lo, hi = bounds[c], bounds[c + 1]
        idxl = np.full((R,), -1.0, dtype=np.float32)
        idxl[: hi - lo] = (gidx[lo:hi] - c * GPC).astype(np.float32)
        m = dict(shared)
        m["atom_r"] = shard(atom, 2 * N_AF, c)
        m["rdf_r"] = shard(rdf, 128, c)
        m["bdf_r"] = shard(bdf, 128, c)
        m["idx_r"] = np.ascontiguousarray(
            idxl.reshape(NSB, 8, 128).transpose(0, 2, 1)
        )
        m["recip_c"] = recip[c * GPC : (c + 1) * GPC].reshape(GPC, 1)
        in_maps.append(m)

    params = dict(E=E, G=G, GPC=GPC, R=R, NSB=NSB, T_pad=T_pad)
    return params, in_maps


# ---------------------------------------------------------------------------
# device program
# ---------------------------------------------------------------------------


def _build(nc, p, fake_collectives=False):
    E, G, GPC, R, NSB = p["E"], p["G"], p["GPC"], p["R"], p["NSB"]
    T_pad = p["T_pad"]
    add = mybir.AluOpType.add
    sub = mybir.AluOpType.subtract
    mult = mybir.AluOpType.mult
    mn = mybir.AluOpType.min
    mx = mybir.AluOpType.max
    iseq = mybir.AluOpType.is_equal
    Relu = mybir.ActivationFunctionType.Relu
    Ident = mybir.ActivationFunctionType.Identity
    Square = mybir.ActivationFunctionType.Square
    Sqrt = mybir.ActivationFunctionType.Sqrt
    AX = mybir.AxisListType.X

    # ---- I/O -------------------------------------------------------------
    atom_d = nc.dram_tensor("atom_r", [NSB, 128, 8, 256], F32, kind="ExternalInput")
    rdf_d = nc.dram_tensor("rdf_r", [NSB, 128, 8, 128], F32, kind="ExternalInput")
    bdf_d = nc.dram_tensor("bdf_r", [NSB, 128, 8, 128], F32, kind="ExternalInput")
    idx_d = nc.dram_tensor("idx_r", [NSB, 128, 8], F32, kind="ExternalInput")
    recip_d = nc.dram_tensor("recip_c", [GPC, 1], F32, kind="ExternalInput")
    wea_d = nc.dram_tensor("w_ea", [128, 128], BF, kind="ExternalInput")
    wer_d = nc.dram_tensor("w_er", [128, 128], BF, kind="ExternalInput")
    web_d = nc.dram_tensor("w_eb", [128, 128], BF, kind="ExternalInput")
    wfr_d = nc.dram_tensor("w_fr", [128, 3, 128], BF, kind="ExternalInput")
    wfb_d = nc.dram_tensor("w_fb", [128, 3, 128], BF, kind="ExternalInput")
    bea_d = nc.dram_tensor("b_ea_c", [128, 1], F32, kind="ExternalInput")
    ber_d = nc.dram_tensor("b_er_c", [128, 1], F32, kind="ExternalInput")
    beb_d = nc.dram_tensor("b_eb_c", [128, 1], F32, kind="ExternalInput")
    bfr_d = nc.dram_tensor("b_fr_c", [128, 1], F32, kind="ExternalInput")
    bfb_d = nc.dram_tensor("b_fb_c", [128, 1], F32, kind="ExternalInput")
    gam_d = nc.dram_tensor("gam_fm", [128, 2], F32, kind="ExternalInput")
    bet_d = nc.dram_tensor("bet_fm", [128, 2], F32, kind="ExternalInput")
    hpad_d = nc.dram_tensor("hpad_fm", [128, 2], F32, kind="ExternalInput")
    hpad2_d = nc.dram_tensor("hpad2_fm", [128, 2], F32, kind="ExternalInput")
    iota_d = nc.dram_tensor("iota_t", [128, 128], F32, kind="ExternalInput")
    ones_d = nc.dram_tensor("ones_r", [1, 128], BF, kind="ExternalInput")
    w1c_d = nc.dram_tensor("w1c", [128, 2, 16], BF, kind="ExternalInput")
    w1r_d = nc.dram_tensor("w1r", [1, 16], BF, kind="ExternalInput")
    b1_d = nc.dram_tensor("b1_c", [16, 1], F32, kind="ExternalInput")
    w2_d = nc.dram_tensor("w2", [16, 1], BF, kind="ExternalInput")
    b2_d = nc.dram_tensor("b2_c", [1, 1], F32, kind="ExternalInput")
    refT_d = nc.dram_tensor("refT", [1, G], BF, kind="ExternalInput")
    out_d = nc.dram_tensor("gap_t", [1, G], F32, kind="ExternalOutput")

    with tile.TileContext(nc) as tc:
        import contextlib

        with contextlib.ExitStack() as S:
            consts = S.enter_context(tc.tile_pool(name="consts", bufs=1))
            statsp = S.enter_context(tc.tile_pool(name="stats", bufs=1))
            dram = S.enter_context(tc.tile_pool(name="dram", bufs=1, space="DRAM"))

            # persistent DRAM scratch
            hb0 = dram.tile([128, R], BF, tag="hb0")
            hb1 = dram.tile([128, R], BF, tag="hb1")
            st_in = dram.tile([128, 4], F32, tag="st_in")
            st_out = dram.tile([128, 4], F32, tag="st_out")
            scl_d = dram.tile([256], F32, tag="scl_d")
            bia_d = dram.tile([256], F32, tag="bia_d")
            ag_in = dram.tile([GPC, 256], BF, tag="ag_in")
            ag_out = dram.tile([G, 256], BF, tag="ag_out")

            # constants in SBUF
            def cload(dt_, handle, shape, name):
                t = consts.tile(shape, dt_, tag=name)
                nc.sync.dma_start(t[:], handle[:])
                return t

            wea = cload(BF, wea_d, [128, 128], "wea")
            wer = cload(BF, wer_d, [128, 128], "wer")
            web = cload(BF, web_d, [128, 128], "web")
            wfr = cload(BF, wfr_d, [128, 3, 128], "wfr")
            wfb = cload(BF, wfb_d, [128, 3, 128], "wfb")
            bea = cload(F32, bea_d, [128, 1], "bea")
            ber = cload(F32, ber_d, [128, 1], "ber")
            beb = cload(F32, beb_d, [128, 1], "beb")
            bfr = cload(F32, bfr_d, [128, 1], "bfr")
            bfb = cload(F32, bfb_d, [128, 1], "bfb")
            gam = cload(F32, gam_d, [128, 2], "gam")
            bet = cload(F32, bet_d, [128, 2], "bet")
            hpad = cload(F32, hpad_d, [128, 2], "hpad")
            hpad2 = cload(F32, hpad2_d, [128, 2], "hpad2")
            iota = cload(F32, iota_d, [128, 128], "iota")
            ones_r = cload(BF, ones_d, [1, 128], "ones")
            w1c = cload(BF, w1c_d, [128, 2, 16], "w1c")
            w1r = cload(BF, w1r_d, [1, 16], "w1r")
            b1c = cload(F32, b1_d, [16, 1], "b1c")
            w2 = cload(BF, w2_d, [16, 1], "w2")
            b2c = cload(F32, b2_d, [1, 1], "b2c")
            refT = cload(BF, refT_d, [1, G], "refT")
            recip = cload(F32, recip_d, [GPC, 1], "recip")

            # BN affine broadcast tiles (filled after the stats AllReduce)
            scaleB = consts.tile([128, 256], BF, tag="scaleB")
            biasB = consts.tile([128, 256], BF, tag="biasB")
            out_sb = consts.tile([1, G], F32, tag="out_sb")

            # per-block stat partials
            ssum0 = statsp.tile([128, 2 * NSB], F32, tag="ssum0")
            ssum1 = statsp.tile([128, 2 * NSB], F32, tag="ssum1")
            ssq0 = statsp.tile([128, 2 * NSB], F32, tag="ssq0")
            ssq1 = statsp.tile([128, 2 * NSB], F32, tag="ssq1")

            # ================= PASS 1 =================
            with contextlib.ExitStack() as S1:
                pAf = S1.enter_context(tc.tile_pool(name="p1_af", bufs=2))
                pRf = S1.enter_context(tc.tile_pool(name="p1_rf", bufs=2))
                pAb = S1.enter_context(tc.tile_pool(name="p1_ab", bufs=2))
                pT = S1.enter_context(tc.tile_pool(name="p1_t", bufs=2))
                pE = S1.enter_context(tc.tile_pool(name="p1_e", bufs=2))
                pS = S1.enter_context(tc.tile_pool(name="p1_s", bufs=3))
                ps1 = S1.enter_context(
                    tc.tile_pool(name="ps_l1", bufs=4, space="PSUM")
                )
                ps2 = S1.enter_context(
                    tc.tile_pool(name="ps_l2", bufs=3, space="PSUM")
                )

                for sb in range(NSB):
                    af = pAf.tile([128, 8, 256], F32, tag="af")
                    rf = pRf.tile([128, 8, 128], F32, tag="rf")
                    bf = pRf.tile([128, 8, 128], F32, tag="bf")
                    nc.sync.dma_start(af[:], atom_d[sb])
                    nc.sync.dma_start(rf[:], rdf_d[sb])
                    nc.sync.dma_start(bf[:], bdf_d[sb])

                    ab = pAb.tile([128, 8, 256], BF, tag="ab")
                    rb = pAb.tile([128, 8, 128], BF, tag="rb")
                    bb = pAb.tile([128, 8, 128], BF, tag="bb")
                    nc.gpsimd.tensor_copy(ab[:], af[:])
                    nc.gpsimd.tensor_copy(rb[:], rf[:])
                    nc.gpsimd.tensor_copy(bb[:], bf[:])

                    # transpose to feature-major [feat, rows]
                    aT = pT.tile([128, 2, 8, 128], BF, tag="aT")
                    rT = pT.tile([128, 8, 128], BF, tag="rT")
                    bT = pT.tile([128, 8, 128], BF, tag="bT")
                    for g in range(8):
                        for h in range(2):
                            nc.sync.dma_start(
                                aT[:, h, g, :],
                                ab[:, g, h * 128 : (h + 1) * 128],
                                transpose=True,
                            )
                        nc.sync.dma_start(rT[:, g, :], rb[:, g, :], transpose=True)
                        nc.sync.dma_start(bT[:, g, :], bb[:, g, :], transpose=True)

                    for blk in range(2):
                        gs = slice(blk * 4, blk * 4 + 4)
                        pa1 = ps1.tile([128, 512], F32, tag="l1")
                        pa2 = ps1.tile([128, 512], F32, tag="l1")
                        pr = ps1.tile([128, 512], F32, tag="l1")
                        pb = ps1.tile([128, 512], F32, tag="l1")
                        nc.tensor.matmul(pa1[:], wea[:], aT[:, 0, gs, :])
                        nc.tensor.matmul(pa2[:], wea[:], aT[:, 1, gs, :])
                        nc.tensor.matmul(pr[:], wer[:], rT[:, gs, :])
                        nc.tensor.matmul(pb[:], web[:], bT[:, gs, :])

                        a1 = pE.tile([128, 512], BF, tag="a1")
                        a2 = pE.tile([128, 512], BF, tag="a2")
                        xr = pE.tile([128, 512], BF, tag="xr")
                        xb = pE.tile([128, 512], BF, tag="xb")
                        t6r = pE.tile([128, 512], BF, tag="t6r")
                        t6b = pE.tile([128, 512], BF, tag="t6b")
                        nc.scalar.activation(a1[:], pa1[:], Relu, bias=bea[:])
                        nc.scalar.activation(a2[:], pa2[:], Relu, bias=bea[:])
                        nc.vector.tensor_scalar(t6r[:], pr[:], ber[:], 6.0, add, mn)
                        nc.vector.tensor_scalar(xr[:], t6r[:], 0.0, None, mx)
                        nc.vector.tensor_scalar(t6b[:], pb[:], beb[:], 6.0, add, mn)
                        nc.vector.tensor_scalar(xb[:], t6b[:], 0.0, None, mx)

                        pfr = ps2.tile([128, 512], F32, tag="l2")
                        pfb = ps2.tile([128, 512], F32, tag="l2")
                        for j, src in enumerate([a1, a2, xr]):
                            nc.tensor.matmul(
                                pfr[:], wfr[:, j, :], src[:],
                                start=(j == 0), stop=(j == 2),
                            )
                        for j, src in enumerate([a1, a2, xb]):
                            nc.tensor.matmul(
                                pfb[:], wfb[:, j, :], src[:],
                                start=(j == 0), stop=(j == 2),
                            )

                        col = 2 * sb + blk
                        sp0 = pS.tile([128, 512], BF, tag="sp0")
                        sp1 = pS.tile([128, 512], BF, tag="sp1")
                        sq0 = pS.tile([128, 512], BF, tag="sq0")
                        sq1 = pS.tile([128, 512], BF, tag="sq1")
                        nc.scalar.activation(
                            sp0[:], pfr[:], Ident, bias=bfr[:],
                            accum_out=ssum0[:, col : col + 1],
                        )
                        nc.scalar.activation(
                            sq0[:], pfr[:], Square, bias=bfr[:],
                            accum_out=ssq0[:, col : col + 1],
                        )
                        nc.scalar.activation(
                            sp1[:], pfb[:], Ident, bias=bfb[:],
                            accum_out=ssum1[:, col : col + 1],
                        )
                        nc.scalar.activation(
                            sq1[:], pfb[:], Square, bias=bfb[:],
                            accum_out=ssq1[:, col : col + 1],
                        )
                        off = sb * 1024 + blk * 512
                        nc.sync.dma_start(hb0[:, off : off + 512], sp0[:])
                        nc.sync.dma_start(hb1[:, off : off + 512], sp1[:])

            # ================= STATS + ALLREDUCE =================
            with contextlib.ExitStack() as SS:
                sp = SS.enter_context(tc.tile_pool(name="stats2", bufs=1))
                psb = SS.enter_context(
                    tc.tile_pool(name="ps_bc", bufs=1, space="PSUM")
                )
                stat = sp.tile([128, 4], F32, tag="stat")
                nc.vector.tensor_reduce(stat[:, 0:1], ssum0[:], AX, add)
                nc.vector.tensor_reduce(stat[:, 1:2], ssum1[:], AX, add)
                nc.vector.tensor_reduce(stat[:, 2:3], ssq0[:], AX, add)
                nc.vector.tensor_reduce(stat[:, 3:4], ssq1[:], AX, add)
                nc.sync.dma_start(st_in[:], stat[:])
                if fake_collectives:
                    nc.gpsimd.dma_start(st_out[:], st_in[:])
                else:
                    nc.gpsimd.collective_compute(
                        "AllReduce",
                        add,
                        replica_groups=[list(range(N_CORES))],
                        ins=[st_in.opt()],
                        outs=[st_out.opt()],
                    )
                statg = sp.tile([128, 4], F32, tag="statg")
                nc.sync.dma_start(statg[:], st_out[:])

                # mu = S/E - hpad*(T_pad/E); msq = S2/E - hpad2*(T_pad/E)
                t1 = sp.tile([128, 2], F32, tag="t1")
                t2 = sp.tile([128, 2], F32, tag="t2")
                muv = sp.tile([128, 2], F32, tag="muv")
                msq = sp.tile([128, 2], F32, tag="msq")
                var = sp.tile([128, 2], F32, tag="var")
                sd = sp.tile([128, 2], F32, tag="sd")
                rstd = sp.tile([128, 2], F32, tag="rstd")
                scl = sp.tile([128, 2], F32, tag="scl")
                bia = sp.tile([128, 2], F32, tag="bia")
                nc.vector.tensor_scalar(t1[:], statg[:, 0:2], 1.0 / E, None, mult)
                nc.vector.tensor_scalar(t2[:], hpad[:], T_pad / E, None, mult)
                nc.vector.tensor_tensor(muv[:], t1[:], t2[:], sub)
                nc.vector.tensor_scalar(t1[:], statg[:, 2:4], 1.0 / E, None, mult)
                nc.vector.tensor_scalar(t2[:], hpad2[:], T_pad / E, None, mult)
                nc.vector.tensor_tensor(msq[:], t1[:], t2[:], sub)
                nc.vector.tensor_tensor(t1[:], muv[:], muv[:], mult)
                nc.vector.tensor_tensor(var[:], msq[:], t1[:], sub)
                eps_t = sp.tile([128, 1], F32, tag="eps")
                nc.vector.memset(eps_t[:], EPS)
                nc.scalar.activation(sd[:], var[:], Sqrt, bias=eps_t[:])
                nc.vector.reciprocal(rstd[:], sd[:])
                nc.vector.tensor_tensor(scl[:], gam[:], rstd[:], mult)
                nc.vector.tensor_tensor(t2[:], muv[:], scl[:], mult)
                nc.vector.tensor_tensor(bia[:], bet[:], t2[:], sub)

                # feature-major [128,2] -> DRAM [256] -> row [1,256] -> bcast
                nc.sync.dma_start(scl_d.opt().rearrange("(j p) -> p j", p=128), scl[:])
                nc.sync.dma_start(bia_d.opt().rearrange("(j p) -> p j", p=128), bia[:])
                srow_f = sp.tile([1, 256], F32, tag="srow_f")
                brow_f = sp.tile([1, 256], F32, tag="brow_f")
                nc.sync.dma_start(srow_f[:], scl_d.opt().rearrange("(j f) -> j f", j=1))
                nc.sync.dma_start(brow_f[:], bia_d.opt().rearrange("(j f) -> j f", j=1))
                srow = sp.tile([1, 256], BF, tag="srow")
                brow = sp.tile([1, 256], BF, tag="brow")
                nc.vector.tensor_copy(srow[:], srow_f[:])
                nc.vector.tensor_copy(brow[:], brow_f[:])
                pbc = psb.tile([128, 256], F32, tag="bc")
                nc.tensor.matmul(pbc[:], ones_r[:], srow[:])
                nc.scalar.activation(scaleB[:], pbc[:], mybir.ActivationFunctionType.Copy)
                pbc2 = psb.tile([128, 256], F32, tag="bc")
                nc.tensor.matmul(pbc2[:], ones_r[:], brow[:])
                nc.scalar.activation(biasB[:], pbc2[:], mybir.ActivationFunctionType.Copy)

            # ================= PASS 2 =================
            with contextlib.ExitStack() as S2:
                p2h = S2.enter_context(tc.tile_pool(name="p2_h", bufs=4))
                p2b = S2.enter_context(tc.tile_pool(name="p2_b", bufs=4))
                p2i = S2.enter_context(tc.tile_pool(name="p2_i", bufs=2))
                psg = S2.enter_context(
                    tc.tile_pool(name="ps_seg", bufs=1, space="PSUM")
                )
                seg = psg.tile([GPC, 256], F32, tag="seg")
                n_groups = NSB * 8
                gi = 0
                for sb in range(NSB):
                    idxt = p2i.tile([128, 8], F32, tag="idx")
                    nc.sync.dma_start(idxt[:], idx_d[sb])
                    for g in range(8):
                        off = sb * 1024 + g * 128
                        hrm = p2h.tile([128, 256], BF, tag="hrm")
                        nc.sync.dma_start(
                            hrm[:, 0:128], hb0[:, off : off + 128], transpose=True
                        )
                        nc.sync.dma_start(
                            hrm[:, 128:256], hb1[:, off : off + 128], transpose=True
                        )
                        tt = p2b.tile([128, 256], BF, tag="tt")
                        uu = p2b.tile([128, 256], BF, tag="uu")
                        hn = p2b.tile([128, 256], BF, tag="hn")
                        oh = p2b.tile([128, GPC], BF, tag="oh")
                        nc.vector.tensor_tensor(tt[:], hrm[:], scaleB[:], mult)
                        nc.vector.tensor_tensor(uu[:], tt[:], biasB[:], add)
                        nc.gpsimd.tensor_scalar(hn[:], uu[:], 0.0, None, mx)
                        nc.gpsimd.tensor_scalar(
                            oh[:], iota[:], idxt[:, g : g + 1], None, iseq
                        )
                        nc.tensor.matmul(
                            seg[:], oh[:], hn[:],
                            start=(gi == 0), stop=(gi == n_groups - 1),
                        )
                        gi += 1

                # pooled means + AllGather
                pooled = p2h.tile([GPC, 256], BF, tag="pooled")
                nc.vector.tensor_scalar(pooled[:], seg[:], recip[:], None, mult)
                nc.sync.dma_start(ag_in[:], pooled[:])
                if fake_collectives:
                    for _k in range(N_CORES):
                        nc.gpsimd.dma_start(
                            ag_out[_k * GPC : (_k + 1) * GPC, :], ag_in[:]
                        )
                else:
                    nc.gpsimd.collective_compute(
                        "AllGather",
                        mybir.AluOpType.bypass,
                        replica_groups=[list(range(N_CORES))],
                        ins=[ag_in.opt()],
                        outs=[ag_out.opt()],
                    )

            # ================= FINAL MLP (replicated) =================
            with contextlib.ExitStack() as SF:
                fp = SF.enter_context(tc.tile_pool(name="fin", bufs=1))
                psf = SF.enter_context(
                    tc.tile_pool(name="ps_fin", bufs=2, space="PSUM")
                )
                nblk = G // 128
                pT0 = fp.tile([128, nblk, 128], BF, tag="pT0")
                pT1 = fp.tile([128, nblk, 128], BF, tag="pT1")
                for k in range(nblk):
                    nc.sync.dma_start(
                        pT0[:, k, :],
                        ag_out[k * 128 : (k + 1) * 128, 0:128],
                        transpose=True,
                    )
                    nc.sync.dma_start(
                        pT1[:, k, :],
                        ag_out[k * 128 : (k + 1) * 128, 128:256],
                        transpose=True,
                    )
                zt = fp.tile([16, G], BF, tag="zt")
                nh = G // 512
                for hh in range(nh):
                    zp = psf.tile([16, 512], F32, tag="zp")
                    gsl = slice(hh * 4, hh * 4 + 4)
                    nc.tensor.matmul(
                        zp[:], w1c[:, 0, :], pT0[:, gsl, :], start=True, stop=False
                    )
                    nc.tensor.matmul(
                        zp[:], w1c[:, 1, :], pT1[:, gsl, :], start=False, stop=False
                    )
                    nc.tensor.matmul(
                        zp[:], w1r[:], refT[:, hh * 512 : (hh + 1) * 512],
                        start=False, stop=True,
                    )
                    nc.scalar.activation(
                        zt[:, hh * 512 : (hh + 1) * 512], zp[:], Relu, bias=b1c[:]
                    )
                for hh in range(nh):
                    gp = psf.tile([1, 512], F32, tag="gp")
                    nc.tensor.matmul(gp[:], w2[:], zt[:, hh * 512 : (hh + 1) * 512])
                    nc.scalar.activation(
                        out_sb[:, hh * 512 : (hh + 1) * 512], gp[:], Relu, bias=b2c[:]
                    )
                nc.sync.dma_start(out_d[:], out_sb[:])

    return nc


# ---------------------------------------------------------------------------
# entry point
# ---------------------------------------------------------------------------

_cache = {}


def _get_program(params_key, params):
    if params_key not in _cache:
        nc = bacc.Bacc(
            "TRN2", target_bir_lowering=False, debug=False, num_devices=N_CORES
        )
        _build(nc, params)
        nc.compile()
        _cache[params_key] = nc
    return _cache[params_key]


def kernel(**inputs) -> np.ndarray:
    params, in_maps = _host_prep(inputs)
    nc = _get_program((params["R"], params["G"], params["E"]), params)
    res = run_bass_kernel_spmd(nc, in_maps, core_ids=list(range(N_CORES)))
    gap = np.asarray(res.results[0]["gap_t"], dtype=np.float32).reshape(-1, 1)
    return gap
